# revision 10
# baseline (speedup 1.0000x reference)
"""Chamfer + edge + normal-cosine combined loss on 8 Trainium2 cores.

v2: candidate-pruned scan. Host kd-sorts both point sets per batch, computes
bbox lower bounds LB(t-tile, p-group) and per-point upper bounds on nearest
distances, and keeps only (tile, group) pairs that can contain a row-min
(LB <= UB_tile) or a column-min (LB <= UB_group) -- provably covering every
exact row/column argmin (~19% of all pairs survive). Kept fine groups (16 pts)
are packed into dense 512-column chunks; the device is a pure streaming
scanner: per step, four K=24 row-tiled bf16 matmuls (tile_position row groups
32j, bf16 3-way-split factors reproduce fp32-accurate dot products) fill one
[128, 2048] PSUM buffer, ACT/DVE alternate casting PSUM->bf16 staging, and the
chunk values ship to DRAM on two DMA queues. Host finishes: row/col maxes over
the shipped bf16 chunks select argmins, final distances are recomputed exactly,
and the tiny edge/normal-cosine terms run in numpy as before.
"""

from contextlib import ExitStack

import ml_dtypes
import numpy as np

B = 4
N = 8192
NCORES = 8
TIL = 128            # t rows per tile (partition dim)
GRP = 16             # fine p-group size for pruning
CHW = 512            # chunk width in columns
GPC = CHW // GRP     # 32 fine groups per chunk
NT = N // TIL        # 64 t-tiles per batch
NGR = N // GRP       # 512 fine groups per batch
KS = 24              # bf16 split rows (3-way split, as baseline)
NNEAR = 3            # groups sampled for upper bounds

_LAST_RESULTS = {}


# ---------------------------------------------------------------- host: split
def _split3(x):
    h = x.astype(ml_dtypes.bfloat16)
    r1 = x - h.astype(np.float32)
    m = r1.astype(ml_dtypes.bfloat16)
    r2 = r1 - m.astype(np.float32)
    l = r2.astype(ml_dtypes.bfloat16)
    return h, m, l


def _build_split_rows(L, R):
    """L [5, X], R [5, Y] fp32 term rows -> bf16 [24, X], [24, Y].

    M = sum_k L[k] (outer) R[k] = 2<g,p> - |g|^2 - |p|^2 = -P."""
    outL, outR = [], []
    for c in range(3):
        Lh, Lm, Ll = _split3(L[c])
        Rh, Rm, Rl = _split3(R[c])
        for a, b in ((Lh, Rh), (Lh, Rm), (Lm, Rh), (Lh, Rl), (Ll, Rh), (Lm, Rm)):
            outL.append(a)
            outR.append(b)
    Xh, Xm, Xl = _split3(L[3])
    negone = R[3].astype(ml_dtypes.bfloat16)
    for a in (Xh, Xm, Xl):
        outL.append(a)
        outR.append(negone)
    Yh, Ym, Yl = _split3(R[4])
    one = L[4].astype(ml_dtypes.bfloat16)
    for b in (Yh, Ym, Yl):
        outL.append(one)
        outR.append(b)
    return np.ascontiguousarray(np.stack(outL)), np.ascontiguousarray(np.stack(outR))


# -------------------------------------------------------------- host: pruning
def _kd_order(pts, leaf):
    """Balanced kd-tree order: median split on widest axis down to `leaf`."""
    out = []
    stack = [np.arange(len(pts))]
    while stack:
        ids = stack.pop()
        if len(ids) <= leaf:
            out.append(ids)
            continue
        p = pts[ids]
        ax = int((p.max(0) - p.min(0)).argmax())
        k = len(ids) // 2
        o = np.argpartition(p[:, ax], k)
        stack.append(ids[o[k:]])
        stack.append(ids[o[:k]])
    # stack order: first-pushed-last; rebuild in left-to-right order
    return np.concatenate(out)


def _point_ubs(A, Btiles, nnear):
    """For each point in A [n,3]: an achievable nearest-distance^2 upper bound,
    the min over all points of the `nnear` nearest B-tiles (by center)."""
    bc = Btiles.mean(1)
    d = ((A[:, None, :] - bc[None, :, :]) ** 2).sum(-1)
    near = np.argpartition(d, nnear, axis=1)[:, :nnear]
    ub = np.full(len(A), np.inf)
    for j in range(nnear):
        sel = near[:, j]
        for g in np.unique(sel):
            m = sel == g
            dd = ((A[m][:, None, :] - Btiles[g][None, :, :]) ** 2).sum(-1).min(1)
            ub[m] = np.minimum(ub[m], dd)
    return ub


def _prep_batch(preds_b, gts_b):
    """Returns sorted perms, per-tile candidate chunk lists and split tables."""
    po = _kd_order(preds_b, GRP)
    go = _kd_order(gts_b, TIL)
    Ps = preds_b[po].astype(np.float64)
    Gs = gts_b[go].astype(np.float64)

    Pt = Ps.reshape(NGR, GRP, 3)
    Gt = Gs.reshape(NT, TIL, 3)
    plo, phi = Pt.min(1), Pt.max(1)
    glo, ghi = Gt.min(1), Gt.max(1)
    d1 = np.maximum(0.0, plo[None, :, :] - ghi[:, None, :])
    d2 = np.maximum(0.0, glo[:, None, :] - phi[None, :, :])
    LB = (np.maximum(d1, d2) ** 2).sum(-1)           # [NT, NGR]

    ub_t = _point_ubs(Gs, Pt, NNEAR)
    UB_T = ub_t.reshape(NT, TIL).max(1)              # [NT]
    ub_p = _point_ubs(Ps, Gt, NNEAR)
    UB_G = ub_p.reshape(NGR, GRP).max(1)             # [NGR]

    keep = LB <= np.maximum(UB_T[:, None], UB_G[None, :]) * (1.0 + 1e-6) + 1e-12

    # chunk lists: per tile, its fine groups packed into CHW-wide chunks
    chunks = []                                      # (tile, group_ids[GPC])
    for T in range(NT):
        gl = np.nonzero(keep[T])[0]
        padded = ((len(gl) + GPC - 1) // GPC) * GPC
        gl = np.resize(gl, padded)      # cycles values to pad
        for c in range(len(gl) // GPC):
            chunks.append((T, gl[c * GPC:(c + 1) * GPC]))

    # split tables over sorted points
    xsq = (Gs * Gs).sum(-1).astype(np.float32)
    ysq = (Ps * Ps).sum(-1).astype(np.float32)
    L = np.empty((5, N), np.float32)
    L[0:3] = (2.0 * Gs.T).astype(np.float32)
    L[3] = xsq
    L[4] = 1.0
    R = np.empty((5, N), np.float32)
    R[0:3] = Ps.T.astype(np.float32)
    R[3] = -1.0
    R[4] = -ysq
    sL, sR = _build_split_rows(L, R)                 # [24, N] bf16 each
    return dict(po=po, go=go, chunks=chunks, sL=sL, sR=sR)


def _prep(preds, gts):
    metas = [_prep_batch(preds[b], gts[b]) for b in range(B)]
    # distribute chunks to cores: core = 2*b + (tile >= 32)
    raw = []
    for b in range(B):
        for h in range(2):
            lo, hi = h * 32, (h + 1) * 32
            raw.append([ch for ch in metas[b]['chunks'] if lo <= ch[0] < hi])
    nreal = [len(cc) for cc in raw]
    steps = max((n + 3) // 4 for n in nreal)
    in_maps = []
    core_data = []
    for c in range(NCORES):
        b = c // 2
        sL, sR = metas[b]['sL'], metas[b]['sR']
        cc = list(raw[c])
        while len(cc) < steps * 4:
            cc.append(cc[-1])
        in0 = np.zeros((steps, 128, 640), ml_dtypes.bfloat16)
        colmap = np.empty((steps * 4, CHW), np.int32)
        tileof = np.empty(steps * 4, np.int32)
        for i, (T, gl) in enumerate(cc):
            cols = (gl[:, None] * GRP + np.arange(GRP)[None, :]).ravel()
            s, j = divmod(i, 4)
            in0[s, 32 * j:32 * j + KS, 0:128] = sL[:, T * TIL:(T + 1) * TIL]
            in0[s, 32 * j:32 * j + KS, 128:640] = sR[:, cols]
            colmap[i] = cols
            tileof[i] = T
        in_maps.append({"in0": in0})
        core_data.append((cc, colmap, tileof))
    return metas, core_data, nreal, steps, in_maps


# ------------------------------------------------------------------- device
def _build_nc(steps):
    import concourse.mybir as mybir
    import concourse.tile as tile
    from concourse import bacc

    f32 = mybir.dt.float32
    bf16 = mybir.dt.bfloat16
    nc = bacc.Bacc("TRN2", target_bir_lowering=False, debug=False)

    in0_d = nc.dram_tensor("in0", [steps, 128, 640], bf16, kind="ExternalInput")
    out_d = nc.dram_tensor("out", [steps, 128, 2048], bf16, kind="ExternalOutput")

    with tile.TileContext(nc) as tc, ExitStack() as ctx:
        io_pool = ctx.enter_context(tc.tile_pool(name="io", bufs=4))
        psum_pool = ctx.enter_context(tc.tile_pool(name="psum", bufs=2, space="PSUM"))
        stage_pool = ctx.enter_context(tc.tile_pool(name="stage", bufs=6))

        for s in range(steps):
            t_in = io_pool.tile([128, 640], bf16)
            nc.scalar.dma_start(t_in[:], in0_d[s, :, :])
            ps = psum_pool.tile([128, 2048], f32, tag="ps")
            for j in range(4):
                nc.tensor.matmul(
                    ps[:, j * 512:(j + 1) * 512],
                    t_in[32 * j:32 * j + KS, 0:128],
                    t_in[32 * j:32 * j + KS, 128:640],
                    start=True,
                    stop=True,
                    tile_position=(32 * j, 0),
                )
            st = stage_pool.tile([128, 2048], bf16, tag="st")
            if s % 2 == 0:
                nc.scalar.copy(st[:], ps[:])
            else:
                nc.vector.tensor_copy(st[:], ps[:])
            (nc.sync if s % 2 == 0 else nc.gpsimd).dma_start(out_d[s, :, :], st[:])

    nc.compile()
    return nc


# ------------------------------------------------------------------ host: post
def _postprocess(preds, gts, normals, edges, results, metas, core_chunks, nreal):
    preds64 = preds.astype(np.float64)
    gts64 = gts.astype(np.float64)

    mins1 = np.empty((B, N), np.float64)
    mins2 = np.empty((B, N), np.float64)
    nearest_idx = np.empty((B, N), np.int64)

    for b in range(B):
        po, go = metas[b]['po'], metas[b]['go']
        # gather both cores' chunk values for this batch
        vals_all, cols_all, tile_all = [], [], []
        for h in range(2):
            c = 2 * b + h
            cc, colmap, tileof = core_chunks[c]
            v = np.asarray(results[c]["out"], ml_dtypes.bfloat16).astype(np.float32)
            v = v.reshape(-1, 128, 4, CHW).transpose(0, 2, 1, 3)
            v = v.reshape(-1, 128, CHW)[:nreal[c]]      # [nch, 128, 512]
            vals_all.append(v)
            cols_all.append(colmap[:nreal[c]])
            tile_all.append(tileof[:nreal[c]])
        vals = np.concatenate(vals_all)                  # [M, 128, 512]
        cols = np.concatenate(cols_all)                  # [M, 512] sorted-p idx
        tils = np.concatenate(tile_all)                  # [M]

        # ---- row path: per tile, max over its chunks' columns
        order = np.argsort(tils, kind='stable')
        vals_o, cols_o, tils_o = vals[order], cols[order], tils[order]
        bounds = np.searchsorted(tils_o, np.arange(NT + 1))
        for T in range(NT):
            i0, i1 = bounds[T], bounds[T + 1]
            v = vals_o[i0:i1]                            # [m, 128, 512]
            flat = v.transpose(1, 0, 2).reshape(TIL, -1)
            am = flat.argmax(1)                          # [128]
            ci, cj = divmod(am, CHW)
            srt_p = cols_o[i0:i1][ci, cj]                # sorted-p index
            t_orig = go[T * TIL + np.arange(TIL)]
            p_orig = po[srt_p]
            d = ((gts64[b, t_orig] - preds64[b, p_orig]) ** 2).sum(-1)
            mins2[b, t_orig] = d
            nearest_idx[b, t_orig] = p_orig

        # ---- col path: per sorted-p column, max over all (chunk, t)
        cmax = vals.max(1)                               # [M, 512]
        cargt = vals.argmax(1)                           # [M, 512] best t-row
        flat_cols = cols.ravel()
        flat_vals = cmax.ravel()
        # global t index of each entry's best row
        trow = (tils[:, None] * TIL + cargt).ravel()     # sorted-t index
        o2 = np.lexsort((-flat_vals, flat_cols))
        fc, first = np.unique(flat_cols[o2], return_index=True)
        assert len(fc) == N, "column coverage hole"
        sel = o2[first]
        srt_t = trow[sel]
        p_orig = po[fc]
        t_orig = go[srt_t]
        d = ((gts64[b, t_orig] - preds64[b, p_orig]) ** 2).sum(-1)
        mins1[b, p_orig] = d

    loss_1 = mins1.mean()
    loss_2 = mins2.mean()
    chamfer = loss_1 + loss_2

    e0 = edges[:, 0]
    e1 = edges[:, 1]
    edge_vectors = preds[:, e0, :] - preds[:, e1, :]
    edge_loss = (edge_vectors * edge_vectors).sum(axis=2).astype(np.float64).mean()

    normals_nearest = np.take_along_axis(normals, nearest_idx[:, :, None], axis=1)
    normals_edge = normals_nearest[:, e0, :]

    def l2n_dim1(v):
        n = np.sqrt((v * v).sum(axis=1, keepdims=True))
        return v / np.maximum(n, 1e-12)

    nn = l2n_dim1(normals_edge)
    nv = l2n_dim1(edge_vectors)
    cosines = np.abs((nn * nv).sum(axis=2))
    normal_cosine_loss = cosines.astype(np.float64).mean()

    return np.float32(
        30000.0 * chamfer + 240.0 * edge_loss + 200000.0 * normal_cosine_loss
    )


def kernel(preds, gts, normals, edges, _trace=False):
    from concourse.bass_utils import run_bass_kernel_spmd

    preds = np.asarray(preds, np.float32)
    gts = np.asarray(gts, np.float32)
    normals = np.asarray(normals, np.float32)
    edges = np.asarray(edges)

    metas, core_data, nreal, steps, in_maps = _prep(preds, gts)
    nc = _build_nc(steps)
    br = run_bass_kernel_spmd(nc, in_maps, list(range(NCORES)), trace=_trace)
    _LAST_RESULTS["bass_results"] = br
    return _postprocess(preds, gts, normals, edges, br.results,
                        metas, core_data, nreal)


# revision 15
# speedup vs baseline: 1.0142x; 1.0142x over previous
"""Chamfer + edge + normal-cosine combined loss on 8 Trainium2 cores.

v2: candidate-pruned scan. Host kd-sorts both point sets per batch, computes
bbox lower bounds LB(t-tile, p-group) and per-point upper bounds on nearest
distances, and keeps only (tile, group) pairs that can contain a row-min
(LB <= UB_tile) or a column-min (LB <= UB_group) -- provably covering every
exact row/column argmin (~19% of all pairs survive). Kept fine groups (16 pts)
are packed into dense 512-column chunks; the device is a pure streaming
scanner: per step, four K=24 row-tiled bf16 matmuls (tile_position row groups
32j, bf16 3-way-split factors reproduce fp32-accurate dot products) fill one
[128, 2048] PSUM buffer, ACT/DVE alternate casting PSUM->bf16 staging, and the
chunk values ship to DRAM on two DMA queues. Host finishes: row/col maxes over
the shipped bf16 chunks select argmins, final distances are recomputed exactly,
and the tiny edge/normal-cosine terms run in numpy as before.
"""

from contextlib import ExitStack

import ml_dtypes
import numpy as np

B = 4
N = 8192
NCORES = 8
TIL = 128            # t rows per tile (partition dim)
GRP = 16             # fine p-group size for pruning
CHW = 512            # chunk width in columns
GPC = CHW // GRP     # 32 fine groups per chunk
NT = N // TIL        # 64 t-tiles per batch
NGR = N // GRP       # 512 fine groups per batch
KS = 24              # bf16 split rows (3-way split, as baseline)
NNEAR = 3            # groups sampled for upper bounds

_LAST_RESULTS = {}


# ---------------------------------------------------------------- host: split
def _split3(x):
    h = x.astype(ml_dtypes.bfloat16)
    r1 = x - h.astype(np.float32)
    m = r1.astype(ml_dtypes.bfloat16)
    r2 = r1 - m.astype(np.float32)
    l = r2.astype(ml_dtypes.bfloat16)
    return h, m, l


def _build_split_rows(L, R):
    """L [5, X], R [5, Y] fp32 term rows -> bf16 [24, X], [24, Y].

    M = sum_k L[k] (outer) R[k] = 2<g,p> - |g|^2 - |p|^2 = -P."""
    outL, outR = [], []
    for c in range(3):
        Lh, Lm, Ll = _split3(L[c])
        Rh, Rm, Rl = _split3(R[c])
        for a, b in ((Lh, Rh), (Lh, Rm), (Lm, Rh), (Lh, Rl), (Ll, Rh), (Lm, Rm)):
            outL.append(a)
            outR.append(b)
    Xh, Xm, Xl = _split3(L[3])
    negone = R[3].astype(ml_dtypes.bfloat16)
    for a in (Xh, Xm, Xl):
        outL.append(a)
        outR.append(negone)
    Yh, Ym, Yl = _split3(R[4])
    one = L[4].astype(ml_dtypes.bfloat16)
    for b in (Yh, Ym, Yl):
        outL.append(one)
        outR.append(b)
    return np.ascontiguousarray(np.stack(outL)), np.ascontiguousarray(np.stack(outR))


# -------------------------------------------------------------- host: pruning
def _kd_order(pts, leaf):
    """Balanced kd-tree order: median split on widest axis down to `leaf`."""
    out = []
    stack = [np.arange(len(pts))]
    while stack:
        ids = stack.pop()
        if len(ids) <= leaf:
            out.append(ids)
            continue
        p = pts[ids]
        ax = int((p.max(0) - p.min(0)).argmax())
        k = len(ids) // 2
        o = np.argpartition(p[:, ax], k)
        stack.append(ids[o[k:]])
        stack.append(ids[o[:k]])
    # stack order: first-pushed-last; rebuild in left-to-right order
    return np.concatenate(out)


def _point_ubs(A, Btiles, nnear):
    """For each point in A [n,3]: an achievable nearest-distance^2 upper bound,
    the min over all points of the `nnear` nearest B-tiles (by center)."""
    bc = Btiles.mean(1)
    d = ((A[:, None, :] - bc[None, :, :]) ** 2).sum(-1)
    near = np.argpartition(d, nnear, axis=1)[:, :nnear]
    ub = np.full(len(A), np.inf)
    for j in range(nnear):
        sel = near[:, j]
        for g in np.unique(sel):
            m = sel == g
            dd = ((A[m][:, None, :] - Btiles[g][None, :, :]) ** 2).sum(-1).min(1)
            ub[m] = np.minimum(ub[m], dd)
    return ub


def _prep_batch(preds_b, gts_b):
    """Returns sorted perms, per-tile candidate chunk lists and split tables."""
    po = _kd_order(preds_b, GRP)
    go = _kd_order(gts_b, TIL)
    Ps = preds_b[po].astype(np.float64)
    Gs = gts_b[go].astype(np.float64)

    Pt = Ps.reshape(NGR, GRP, 3)
    Gt = Gs.reshape(NT, TIL, 3)
    plo, phi = Pt.min(1), Pt.max(1)
    glo, ghi = Gt.min(1), Gt.max(1)
    d1 = np.maximum(0.0, plo[None, :, :] - ghi[:, None, :])
    d2 = np.maximum(0.0, glo[:, None, :] - phi[None, :, :])
    LB = (np.maximum(d1, d2) ** 2).sum(-1)           # [NT, NGR]

    ub_t = _point_ubs(Gs, Pt, NNEAR)
    UB_T = ub_t.reshape(NT, TIL).max(1)              # [NT]
    ub_p = _point_ubs(Ps, Gt, NNEAR)
    UB_G = ub_p.reshape(NGR, GRP).max(1)             # [NGR]

    keep = LB <= np.maximum(UB_T[:, None], UB_G[None, :]) * (1.0 + 1e-6) + 1e-12

    # chunk lists: per tile, its fine groups packed into CHW-wide chunks
    chunks = []                                      # (tile, group_ids[GPC])
    for T in range(NT):
        gl = np.nonzero(keep[T])[0]
        padded = ((len(gl) + GPC - 1) // GPC) * GPC
        gl = np.resize(gl, padded)      # cycles values to pad
        for c in range(len(gl) // GPC):
            chunks.append((T, gl[c * GPC:(c + 1) * GPC]))

    # split tables over sorted points
    xsq = (Gs * Gs).sum(-1).astype(np.float32)
    ysq = (Ps * Ps).sum(-1).astype(np.float32)
    L = np.empty((5, N), np.float32)
    L[0:3] = (2.0 * Gs.T).astype(np.float32)
    L[3] = xsq
    L[4] = 1.0
    R = np.empty((5, N), np.float32)
    R[0:3] = Ps.T.astype(np.float32)
    R[3] = -1.0
    R[4] = -ysq
    sL, sR = _build_split_rows(L, R)                 # [24, N] bf16 each
    return dict(po=po, go=go, chunks=chunks, sL=sL, sR=sR)


def _prep(preds, gts):
    metas = [_prep_batch(preds[b], gts[b]) for b in range(B)]
    # distribute chunks to cores: core = 2*b + (tile >= 32)
    raw = []
    for b in range(B):
        for h in range(2):
            lo, hi = h * 32, (h + 1) * 32
            raw.append([ch for ch in metas[b]['chunks'] if lo <= ch[0] < hi])
    nreal = [len(cc) for cc in raw]
    steps = max((n + 3) // 4 for n in nreal)
    steps += steps % 2          # even, for 2-step DMA batching
    in_maps = []
    core_data = []
    for c in range(NCORES):
        b = c // 2
        sL, sR = metas[b]['sL'], metas[b]['sR']
        cc = list(raw[c])
        while len(cc) < steps * 4:
            cc.append(cc[-1])
        in0 = np.zeros((steps // 2, 128, 1280), ml_dtypes.bfloat16)
        colmap = np.empty((steps * 4, CHW), np.int32)
        tileof = np.empty(steps * 4, np.int32)
        for i, (T, gl) in enumerate(cc):
            cols = (gl[:, None] * GRP + np.arange(GRP)[None, :]).ravel()
            s, j = divmod(i, 4)
            s2, p = divmod(s, 2)
            o = p * 640
            in0[s2, 32 * j:32 * j + KS, o:o + 128] = sL[:, T * TIL:(T + 1) * TIL]
            in0[s2, 32 * j:32 * j + KS, o + 128:o + 640] = sR[:, cols]
            colmap[i] = cols
            tileof[i] = T
        in_maps.append({"in0": in0})
        core_data.append((cc, colmap, tileof))
    return metas, core_data, nreal, steps, in_maps


# ------------------------------------------------------------------- device
def _build_nc(steps):
    import concourse.mybir as mybir
    import concourse.tile as tile
    from concourse import bacc

    f32 = mybir.dt.float32
    bf16 = mybir.dt.bfloat16
    nc = bacc.Bacc("TRN2", target_bir_lowering=False, debug=False)

    # 2 steps batched per DMA: bigger contiguous lines -> fewer DMA packets
    in0_d = nc.dram_tensor("in0", [steps // 2, 128, 1280], bf16, kind="ExternalInput")
    out_d = nc.dram_tensor("out", [steps // 2, 128, 4096], bf16, kind="ExternalOutput")

    with tile.TileContext(nc) as tc, ExitStack() as ctx:
        io_pool = ctx.enter_context(tc.tile_pool(name="io", bufs=3))
        psum_pool = ctx.enter_context(tc.tile_pool(name="psum", bufs=2, space="PSUM"))
        stage_pool = ctx.enter_context(tc.tile_pool(name="stage", bufs=3))

        for s2 in range(steps // 2):
            t_in = io_pool.tile([128, 1280], bf16)
            nc.scalar.dma_start(t_in[:], in0_d[s2, :, :])
            for p in range(2):
                ps = psum_pool.tile([128, 2048], f32, tag="ps")
                for j in range(4):
                    nc.tensor.matmul(
                        ps[:, j * 512:(j + 1) * 512],
                        t_in[32 * j:32 * j + KS, p * 640:p * 640 + 128],
                        t_in[32 * j:32 * j + KS, p * 640 + 128:p * 640 + 640],
                        start=True,
                        stop=True,
                        tile_position=(32 * j, 0),
                    )
                if p == 0:
                    st = stage_pool.tile([128, 2048], bf16, tag="sta")
                    nc.scalar.copy(st[:], ps[:])
                    nc.gpsimd.dma_start(out_d[s2, :, 0:2048], st[:])
                else:
                    st = stage_pool.tile([128, 2048], bf16, tag="stb")
                    nc.vector.tensor_copy(st[:], ps[:])
                    nc.sync.dma_start(out_d[s2, :, 2048:4096], st[:])

    nc.compile()
    return nc


# ------------------------------------------------------------------ host: post
def _postprocess(preds, gts, normals, edges, results, metas, core_chunks, nreal):
    preds64 = preds.astype(np.float64)
    gts64 = gts.astype(np.float64)

    mins1 = np.empty((B, N), np.float64)
    mins2 = np.empty((B, N), np.float64)
    nearest_idx = np.empty((B, N), np.int64)

    for b in range(B):
        po, go = metas[b]['po'], metas[b]['go']
        # gather both cores' chunk values for this batch
        vals_all, cols_all, tile_all = [], [], []
        for h in range(2):
            c = 2 * b + h
            cc, colmap, tileof = core_chunks[c]
            v = np.asarray(results[c]["out"], ml_dtypes.bfloat16).astype(np.float32)
            v = v.reshape(-1, 128, 8, CHW).transpose(0, 2, 1, 3)
            v = v.reshape(-1, 128, CHW)[:nreal[c]]      # [nch, 128, 512]
            vals_all.append(v)
            cols_all.append(colmap[:nreal[c]])
            tile_all.append(tileof[:nreal[c]])
        vals = np.concatenate(vals_all)                  # [M, 128, 512]
        cols = np.concatenate(cols_all)                  # [M, 512] sorted-p idx
        tils = np.concatenate(tile_all)                  # [M]

        # ---- row path: per tile, max over its chunks' columns
        order = np.argsort(tils, kind='stable')
        vals_o, cols_o, tils_o = vals[order], cols[order], tils[order]
        bounds = np.searchsorted(tils_o, np.arange(NT + 1))
        for T in range(NT):
            i0, i1 = bounds[T], bounds[T + 1]
            v = vals_o[i0:i1]                            # [m, 128, 512]
            flat = v.transpose(1, 0, 2).reshape(TIL, -1)
            am = flat.argmax(1)                          # [128]
            ci, cj = divmod(am, CHW)
            srt_p = cols_o[i0:i1][ci, cj]                # sorted-p index
            t_orig = go[T * TIL + np.arange(TIL)]
            p_orig = po[srt_p]
            d = ((gts64[b, t_orig] - preds64[b, p_orig]) ** 2).sum(-1)
            mins2[b, t_orig] = d
            nearest_idx[b, t_orig] = p_orig

        # ---- col path: per sorted-p column, max over all (chunk, t)
        cmax = vals.max(1)                               # [M, 512]
        cargt = vals.argmax(1)                           # [M, 512] best t-row
        flat_cols = cols.ravel()
        flat_vals = cmax.ravel()
        # global t index of each entry's best row
        trow = (tils[:, None] * TIL + cargt).ravel()     # sorted-t index
        o2 = np.lexsort((-flat_vals, flat_cols))
        fc, first = np.unique(flat_cols[o2], return_index=True)
        assert len(fc) == N, "column coverage hole"
        sel = o2[first]
        srt_t = trow[sel]
        p_orig = po[fc]
        t_orig = go[srt_t]
        d = ((gts64[b, t_orig] - preds64[b, p_orig]) ** 2).sum(-1)
        mins1[b, p_orig] = d

    loss_1 = mins1.mean()
    loss_2 = mins2.mean()
    chamfer = loss_1 + loss_2

    e0 = edges[:, 0]
    e1 = edges[:, 1]
    edge_vectors = preds[:, e0, :] - preds[:, e1, :]
    edge_loss = (edge_vectors * edge_vectors).sum(axis=2).astype(np.float64).mean()

    normals_nearest = np.take_along_axis(normals, nearest_idx[:, :, None], axis=1)
    normals_edge = normals_nearest[:, e0, :]

    def l2n_dim1(v):
        n = np.sqrt((v * v).sum(axis=1, keepdims=True))
        return v / np.maximum(n, 1e-12)

    nn = l2n_dim1(normals_edge)
    nv = l2n_dim1(edge_vectors)
    cosines = np.abs((nn * nv).sum(axis=2))
    normal_cosine_loss = cosines.astype(np.float64).mean()

    return np.float32(
        30000.0 * chamfer + 240.0 * edge_loss + 200000.0 * normal_cosine_loss
    )


def kernel(preds, gts, normals, edges, _trace=False):
    from concourse.bass_utils import run_bass_kernel_spmd

    preds = np.asarray(preds, np.float32)
    gts = np.asarray(gts, np.float32)
    normals = np.asarray(normals, np.float32)
    edges = np.asarray(edges)

    metas, core_data, nreal, steps, in_maps = _prep(preds, gts)
    nc = _build_nc(steps)
    br = run_bass_kernel_spmd(nc, in_maps, list(range(NCORES)), trace=_trace)
    _LAST_RESULTS["bass_results"] = br
    return _postprocess(preds, gts, normals, edges, br.results,
                        metas, core_data, nreal)


# revision 17
# speedup vs baseline: 1.1330x; 1.1172x over previous
"""Chamfer + edge + normal-cosine combined loss on 8 Trainium2 cores.

v2: candidate-pruned scan. Host kd-sorts both point sets per batch, computes
bbox lower bounds LB(t-tile, p-group) and per-point upper bounds on nearest
distances, and keeps only (tile, group) pairs that can contain a row-min
(LB <= UB_tile) or a column-min (LB <= UB_group) -- provably covering every
exact row/column argmin (~19% of all pairs survive). Kept fine groups (16 pts)
are packed into dense 512-column chunks; the device is a pure streaming
scanner: per step, four K=24 row-tiled bf16 matmuls (tile_position row groups
32j, bf16 3-way-split factors reproduce fp32-accurate dot products) fill one
[128, 2048] PSUM buffer, ACT/DVE alternate casting PSUM->bf16 staging, and the
chunk values ship to DRAM on two DMA queues. Host finishes: row/col maxes over
the shipped bf16 chunks select argmins, final distances are recomputed exactly,
and the tiny edge/normal-cosine terms run in numpy as before.
"""

from contextlib import ExitStack

import ml_dtypes
import numpy as np

B = 4
N = 8192
NCORES = 8
TIL = 128            # t rows per tile (partition dim)
GRP = 8              # fine p-group size for pruning
CHW = 512            # chunk width in columns
GPC = CHW // GRP     # 32 fine groups per chunk
NT = N // TIL        # 64 t-tiles per batch
NGR = N // GRP       # 512 fine groups per batch
KS = 24              # bf16 split rows (3-way split, as baseline)
NNEAR = 5            # groups sampled for upper bounds

_LAST_RESULTS = {}


# ---------------------------------------------------------------- host: split
def _split3(x):
    h = x.astype(ml_dtypes.bfloat16)
    r1 = x - h.astype(np.float32)
    m = r1.astype(ml_dtypes.bfloat16)
    r2 = r1 - m.astype(np.float32)
    l = r2.astype(ml_dtypes.bfloat16)
    return h, m, l


def _build_split_rows(L, R):
    """L [5, X], R [5, Y] fp32 term rows -> bf16 [24, X], [24, Y].

    M = sum_k L[k] (outer) R[k] = 2<g,p> - |g|^2 - |p|^2 = -P."""
    outL, outR = [], []
    for c in range(3):
        Lh, Lm, Ll = _split3(L[c])
        Rh, Rm, Rl = _split3(R[c])
        for a, b in ((Lh, Rh), (Lh, Rm), (Lm, Rh), (Lh, Rl), (Ll, Rh), (Lm, Rm)):
            outL.append(a)
            outR.append(b)
    Xh, Xm, Xl = _split3(L[3])
    negone = R[3].astype(ml_dtypes.bfloat16)
    for a in (Xh, Xm, Xl):
        outL.append(a)
        outR.append(negone)
    Yh, Ym, Yl = _split3(R[4])
    one = L[4].astype(ml_dtypes.bfloat16)
    for b in (Yh, Ym, Yl):
        outL.append(one)
        outR.append(b)
    return np.ascontiguousarray(np.stack(outL)), np.ascontiguousarray(np.stack(outR))


# -------------------------------------------------------------- host: pruning
def _kd_order(pts, leaf):
    """Balanced kd-tree order: median split on widest axis down to `leaf`."""
    out = []
    stack = [np.arange(len(pts))]
    while stack:
        ids = stack.pop()
        if len(ids) <= leaf:
            out.append(ids)
            continue
        p = pts[ids]
        ax = int((p.max(0) - p.min(0)).argmax())
        k = len(ids) // 2
        o = np.argpartition(p[:, ax], k)
        stack.append(ids[o[k:]])
        stack.append(ids[o[:k]])
    # stack order: first-pushed-last; rebuild in left-to-right order
    return np.concatenate(out)


def _point_ubs(A, Btiles, nnear):
    """For each point in A [n,3]: an achievable nearest-distance^2 upper bound,
    the min over all points of the `nnear` nearest B-tiles (by center)."""
    bc = Btiles.mean(1)
    d = ((A[:, None, :] - bc[None, :, :]) ** 2).sum(-1)
    near = np.argpartition(d, nnear, axis=1)[:, :nnear]
    ub = np.full(len(A), np.inf)
    for j in range(nnear):
        sel = near[:, j]
        for g in np.unique(sel):
            m = sel == g
            dd = ((A[m][:, None, :] - Btiles[g][None, :, :]) ** 2).sum(-1).min(1)
            ub[m] = np.minimum(ub[m], dd)
    return ub


def _prep_batch(preds_b, gts_b):
    """Returns sorted perms, per-tile candidate chunk lists and split tables."""
    po = _kd_order(preds_b, GRP)
    go = _kd_order(gts_b, TIL)
    Ps = preds_b[po].astype(np.float64)
    Gs = gts_b[go].astype(np.float64)

    Pt = Ps.reshape(NGR, GRP, 3)
    Gt = Gs.reshape(NT, TIL, 3)
    plo, phi = Pt.min(1), Pt.max(1)
    glo, ghi = Gt.min(1), Gt.max(1)
    d1 = np.maximum(0.0, plo[None, :, :] - ghi[:, None, :])
    d2 = np.maximum(0.0, glo[:, None, :] - phi[None, :, :])
    LB = (np.maximum(d1, d2) ** 2).sum(-1)           # [NT, NGR]

    ub_t = _point_ubs(Gs, Pt, NNEAR)
    UB_T = ub_t.reshape(NT, TIL).max(1)              # [NT]
    ub_p = _point_ubs(Ps, Gt, NNEAR)
    UB_G = ub_p.reshape(NGR, GRP).max(1)             # [NGR]

    keep = LB <= np.maximum(UB_T[:, None], UB_G[None, :]) * (1.0 + 1e-6) + 1e-12

    # chunk lists: per tile, its fine groups packed into CHW-wide chunks
    chunks = []                                      # (tile, group_ids[GPC])
    for T in range(NT):
        gl = np.nonzero(keep[T])[0]
        padded = ((len(gl) + GPC - 1) // GPC) * GPC
        gl = np.resize(gl, padded)      # cycles values to pad
        for c in range(len(gl) // GPC):
            chunks.append((T, gl[c * GPC:(c + 1) * GPC]))

    # split tables over sorted points
    xsq = (Gs * Gs).sum(-1).astype(np.float32)
    ysq = (Ps * Ps).sum(-1).astype(np.float32)
    L = np.empty((5, N), np.float32)
    L[0:3] = (2.0 * Gs.T).astype(np.float32)
    L[3] = xsq
    L[4] = 1.0
    R = np.empty((5, N), np.float32)
    R[0:3] = Ps.T.astype(np.float32)
    R[3] = -1.0
    R[4] = -ysq
    sL, sR = _build_split_rows(L, R)                 # [24, N] bf16 each
    return dict(po=po, go=go, chunks=chunks, sL=sL, sR=sR)


def _prep(preds, gts):
    metas = [_prep_batch(preds[b], gts[b]) for b in range(B)]
    # distribute chunks to cores: core = 2*b + (tile >= 32)
    raw = []
    for b in range(B):
        for h in range(2):
            lo, hi = h * 32, (h + 1) * 32
            raw.append([ch for ch in metas[b]['chunks'] if lo <= ch[0] < hi])
    nreal = [len(cc) for cc in raw]
    steps = max((n + 3) // 4 for n in nreal)
    steps = ((steps + 3) // 4) * 4   # multiple of 4 for DMA batching
    in_maps = []
    core_data = []
    for c in range(NCORES):
        b = c // 2
        sL, sR = metas[b]['sL'], metas[b]['sR']
        cc = list(raw[c])
        while len(cc) < steps * 4:
            cc.append(cc[-1])
        in0 = np.zeros((steps // 4, 128, 2560), ml_dtypes.bfloat16)
        colmap = np.empty((steps * 4, CHW), np.int32)
        tileof = np.empty(steps * 4, np.int32)
        for i, (T, gl) in enumerate(cc):
            cols = (gl[:, None] * GRP + np.arange(GRP)[None, :]).ravel()
            s, j = divmod(i, 4)
            s4, p = divmod(s, 4)
            o = p * 640
            in0[s4, 32 * j:32 * j + KS, o:o + 128] = sL[:, T * TIL:(T + 1) * TIL]
            in0[s4, 32 * j:32 * j + KS, o + 128:o + 640] = sR[:, cols]
            colmap[i] = cols
            tileof[i] = T
        in_maps.append({"in0": in0})
        core_data.append((cc, colmap, tileof))
    return metas, core_data, nreal, steps, in_maps


# ------------------------------------------------------------------- device
def _build_nc(steps):
    import concourse.mybir as mybir
    import concourse.tile as tile
    from concourse import bacc

    f32 = mybir.dt.float32
    bf16 = mybir.dt.bfloat16
    nc = bacc.Bacc("TRN2", target_bir_lowering=False, debug=False)

    # 2 steps batched per DMA: bigger contiguous lines -> fewer DMA packets
    in0_d = nc.dram_tensor("in0", [steps // 4, 128, 2560], bf16, kind="ExternalInput")
    out_d = nc.dram_tensor("out", [steps // 2, 128, 4096], bf16, kind="ExternalOutput")

    with tile.TileContext(nc) as tc, ExitStack() as ctx:
        io_pool = ctx.enter_context(tc.tile_pool(name="io", bufs=3))
        psum_pool = ctx.enter_context(tc.tile_pool(name="psum", bufs=2, space="PSUM"))
        stage_pool = ctx.enter_context(tc.tile_pool(name="stage", bufs=3))

        for s4 in range(steps // 4):
            t_in = io_pool.tile([128, 2560], bf16)
            nc.scalar.dma_start(t_in[:], in0_d[s4, :, :])
            for p in range(4):
                s = s4 * 4 + p
                s2, half = divmod(s, 2)
                ps = psum_pool.tile([128, 2048], f32, tag="ps")
                for j in range(4):
                    nc.tensor.matmul(
                        ps[:, j * 512:(j + 1) * 512],
                        t_in[32 * j:32 * j + KS, p * 640:p * 640 + 128],
                        t_in[32 * j:32 * j + KS, p * 640 + 128:p * 640 + 640],
                        start=True,
                        stop=True,
                        tile_position=(32 * j, 0),
                    )
                if half == 0:
                    st = stage_pool.tile([128, 2048], bf16, tag="sta")
                    nc.scalar.copy(st[:], ps[:])
                    nc.gpsimd.dma_start(out_d[s2, :, 0:2048], st[:])
                else:
                    st = stage_pool.tile([128, 2048], bf16, tag="stb")
                    nc.vector.tensor_copy(st[:], ps[:])
                    nc.sync.dma_start(out_d[s2, :, 2048:4096], st[:])

    nc.compile()
    return nc


# ------------------------------------------------------------------ host: post
def _postprocess(preds, gts, normals, edges, results, metas, core_chunks, nreal):
    preds64 = preds.astype(np.float64)
    gts64 = gts.astype(np.float64)

    mins1 = np.empty((B, N), np.float64)
    mins2 = np.empty((B, N), np.float64)
    nearest_idx = np.empty((B, N), np.int64)

    for b in range(B):
        po, go = metas[b]['po'], metas[b]['go']
        # gather both cores' chunk values for this batch
        vals_all, cols_all, tile_all = [], [], []
        for h in range(2):
            c = 2 * b + h
            cc, colmap, tileof = core_chunks[c]
            v = np.asarray(results[c]["out"], ml_dtypes.bfloat16).astype(np.float32)
            v = v.reshape(-1, 128, 8, CHW).transpose(0, 2, 1, 3)
            v = v.reshape(-1, 128, CHW)[:nreal[c]]      # [nch, 128, 512]
            vals_all.append(v)
            cols_all.append(colmap[:nreal[c]])
            tile_all.append(tileof[:nreal[c]])
        vals = np.concatenate(vals_all)                  # [M, 128, 512]
        cols = np.concatenate(cols_all)                  # [M, 512] sorted-p idx
        tils = np.concatenate(tile_all)                  # [M]

        # ---- row path: per tile, max over its chunks' columns
        order = np.argsort(tils, kind='stable')
        vals_o, cols_o, tils_o = vals[order], cols[order], tils[order]
        bounds = np.searchsorted(tils_o, np.arange(NT + 1))
        for T in range(NT):
            i0, i1 = bounds[T], bounds[T + 1]
            v = vals_o[i0:i1]                            # [m, 128, 512]
            flat = v.transpose(1, 0, 2).reshape(TIL, -1)
            am = flat.argmax(1)                          # [128]
            ci, cj = divmod(am, CHW)
            srt_p = cols_o[i0:i1][ci, cj]                # sorted-p index
            t_orig = go[T * TIL + np.arange(TIL)]
            p_orig = po[srt_p]
            d = ((gts64[b, t_orig] - preds64[b, p_orig]) ** 2).sum(-1)
            mins2[b, t_orig] = d
            nearest_idx[b, t_orig] = p_orig

        # ---- col path: per sorted-p column, max over all (chunk, t)
        cmax = vals.max(1)                               # [M, 512]
        cargt = vals.argmax(1)                           # [M, 512] best t-row
        flat_cols = cols.ravel()
        flat_vals = cmax.ravel()
        # global t index of each entry's best row
        trow = (tils[:, None] * TIL + cargt).ravel()     # sorted-t index
        o2 = np.lexsort((-flat_vals, flat_cols))
        fc, first = np.unique(flat_cols[o2], return_index=True)
        assert len(fc) == N, "column coverage hole"
        sel = o2[first]
        srt_t = trow[sel]
        p_orig = po[fc]
        t_orig = go[srt_t]
        d = ((gts64[b, t_orig] - preds64[b, p_orig]) ** 2).sum(-1)
        mins1[b, p_orig] = d

    loss_1 = mins1.mean()
    loss_2 = mins2.mean()
    chamfer = loss_1 + loss_2

    e0 = edges[:, 0]
    e1 = edges[:, 1]
    edge_vectors = preds[:, e0, :] - preds[:, e1, :]
    edge_loss = (edge_vectors * edge_vectors).sum(axis=2).astype(np.float64).mean()

    normals_nearest = np.take_along_axis(normals, nearest_idx[:, :, None], axis=1)
    normals_edge = normals_nearest[:, e0, :]

    def l2n_dim1(v):
        n = np.sqrt((v * v).sum(axis=1, keepdims=True))
        return v / np.maximum(n, 1e-12)

    nn = l2n_dim1(normals_edge)
    nv = l2n_dim1(edge_vectors)
    cosines = np.abs((nn * nv).sum(axis=2))
    normal_cosine_loss = cosines.astype(np.float64).mean()

    return np.float32(
        30000.0 * chamfer + 240.0 * edge_loss + 200000.0 * normal_cosine_loss
    )


def kernel(preds, gts, normals, edges, _trace=False):
    from concourse.bass_utils import run_bass_kernel_spmd

    preds = np.asarray(preds, np.float32)
    gts = np.asarray(gts, np.float32)
    normals = np.asarray(normals, np.float32)
    edges = np.asarray(edges)

    metas, core_data, nreal, steps, in_maps = _prep(preds, gts)
    nc = _build_nc(steps)
    br = run_bass_kernel_spmd(nc, in_maps, list(range(NCORES)), trace=_trace)
    _LAST_RESULTS["bass_results"] = br
    return _postprocess(preds, gts, normals, edges, br.results,
                        metas, core_data, nreal)
